# revision 14
# baseline (speedup 1.0000x reference)
"""Trainium2 Bass kernel for CointegrationAttentionLayer.

Reference computation (per batch b, ids = stock_ids[b], X = stock_features[b]):
    G_A[i,j] = attention_weights[ids_i, ids_j]   (0 on i==j diag)
    G_M[i,j] = interaction_matrix[ids_i, ids_j]  (0 on i==j diag)
    w = |G_A|; attn = softmax(w, axis=j)
    out[b] = (G_M * attn) @ X

Algebraic refactor (removes the on-chip column gather, the old bottleneck):
    E = exp(|A|), P = M * E                       (host: parameter folding)
    out[b,i] = (1/Z_i) * (sum_v P[ids_i, v] * XS_b[v]  -  P[ids_i,ids_i] * X[b,i])
    XS_b[v]  = sum_{j: ids_j = v} X[b,j]          (host scatter-add, O(N*F))
    Z[b,i]   = sum_v E[ids_i, v] * count_b[v] - E[ids_i,ids_i] + 1   (host)

Only the v in unique(ids_b) rows of XS_b are nonzero, so the device contracts
over K = 1024 padded unique ids:
    out_vT[f, u] = sum_k XSc_b[k, f] * PTc[U_b[k], u]
The device row-gathers PTc[U_b] via the 16-queue dma_gather (bf16 rows) and
runs dense accumulating matmuls into PSUM; no GpSimd ap_gather at all.
The u axis is compacted per core to the union of its 4 batches' ids
(~2563 of 4000, padded to NPC=2816), cutting gather traffic and matmul
stream length by ~31%.  Softmax normalization, the positional-diagonal
correction and the final row selection out_v[ids_i] are cheap O(B*N*F)
host index math on the returned [f, u] planes.

Sharding: data-parallel, 4 batches per core; per-core column-compacted table.
"""

import numpy as np
import ml_dtypes

import concourse.bacc as bacc
import concourse.bass as bass
import concourse.tile as tile
from concourse import mybir
from concourse.bass_utils import run_bass_kernel_spmd

B, N, F, V = 32, 1024, 128, 4000
NCORES = 8
BPC = B // NCORES    # batches per core
NKT = N // 128       # 8 contraction k-tiles per batch
NB = 512             # PSUM bank width in f32
NPC = 2816           # compacted/padded u-columns per core (22*128)

_prog_cache = {}


def _build_program(npc, counts):
    """counts[bb] = shared-across-cores valid index count for batch slot bb
    (indices beyond it are negative pads that dma_gather skips; the count is
    passed via num_idxs_reg so the decode-side descriptor-ring accounting
    stays in lockstep with the ucode's trailing-negative trim)."""
    key = (npc, tuple(counts))
    if key in _prog_cache:
        return _prog_cache[key]

    f32 = mybir.dt.float32
    bf16 = mybir.dt.bfloat16
    i16 = mybir.dt.int16

    nbanks = [NB] * (npc // NB)
    if npc % NB:
        nbanks.append(npc % NB)

    nc = bacc.Bacc(None, target_bir_lowering=False)
    pt2 = nc.declare_dram_parameter("pt2", [V, npc], bf16, isOutput=False)
    # xsc[b, p, kt*F + f] = XSc_b[kt*128 + p, f]
    xsc = nc.declare_dram_parameter("xsc", [BPC, 128, NKT * F], bf16,
                                    isOutput=False)
    # uidx[b] cols kt*8..kt*8+8 = wrapped int16 unique-id rows of k-tile kt
    uidx = nc.declare_dram_parameter("uidx", [BPC, 128, 64], i16,
                                     isOutput=False)
    out = nc.declare_dram_parameter("out", [BPC, 128, npc], bf16,
                                    isOutput=True)

    with tile.TileContext(nc) as tc, \
            tc.tile_pool(name="gat", bufs=2) as gatp, \
            tc.tile_pool(name="sml", bufs=1) as smlp, \
            tc.tile_pool(name="ob", bufs=4) as obp, \
            tc.tile_pool(name="psum", bufs=1, space="PSUM") as psump:
        uits, xsts = [], []
        for b in range(BPC):
            uit = smlp.tile([128, 64], i16, tag=f"uit{b}", name=f"uit{b}")
            nc.sync.dma_start(out=uit[:], in_=uidx[b])
            uits.append(uit)
            xst = smlp.tile([128, NKT * F], bf16, tag=f"xst{b}",
                            name=f"xst{b}")
            nc.sync.dma_start(out=xst[:], in_=xsc[b])
            xsts.append(xst)

        # Unique-id lists are padded with NEGATIVE indices, which dma_gather
        # skips (no DMA traffic).  Skipped partitions keep their previous
        # SBUF content, which the zero lhsT rows nullify in the matmul -- but
        # the first two generations of each gather tile are uninitialized
        # SBUF (could hold Inf/NaN bit patterns, and 0*NaN = NaN), so memset
        # them once up front on the otherwise-idle Vector engine.
        pregen = []
        for g in range(2):
            row = []
            for kt in range(NKT):
                bt = gatp.tile([128, npc], bf16, tag=f"bt{kt}", name=f"bt{kt}")
                nc.vector.memset(bt[:], 0.0)
                row.append(bt)
            pregen.append(row)

        for b in range(BPC):
            uit, xst = uits[b], xsts[b]
            bts = []
            for kt in range(NKT):
                if b < 2:
                    bt = pregen[b][kt]
                else:
                    bt = gatp.tile([128, npc], bf16, tag=f"bt{kt}",
                                   name=f"bt{kt}")
                nvalid = max(0, min(128, counts[b] - kt * 128))
                nc.gpsimd.dma_gather(
                    out_ap=bt[:].rearrange("p (o e) -> p o e", o=1),
                    in_ap=pt2[:],
                    idxs_ap=uit[:, kt * 8:(kt + 1) * 8],
                    num_idxs=128,
                    num_idxs_reg=nvalid,
                    elem_size=npc,
                )
                bts.append(bt)

            pos = [
                psump.tile([128, w], f32, tag=f"po{h}", name=f"po{h}",
                           space="PSUM")
                for h, w in enumerate(nbanks)
            ]
            for kt in range(NKT):
                for h, w in enumerate(nbanks):
                    nc.tensor.matmul(
                        out=pos[h][:],
                        lhsT=xst[:, kt * F:(kt + 1) * F],
                        rhs=bts[kt][:, h * NB:h * NB + w],
                        start=(kt == 0),
                        stop=(kt == NKT - 1),
                    )
            for h, w in enumerate(nbanks):
                ob = obp.tile([128, w], bf16, tag=f"ob{h}")
                nc.vector.tensor_copy(out=ob[:], in_=pos[h][:])
                nc.scalar.dma_start(out=out[b, :, h * NB:h * NB + w],
                                    in_=ob[:])

    nc.compile()
    _prog_cache[key] = nc
    return nc


def _wrap16(a):
    """[n] int array -> [128, n//16] int16 'wrapped in 16 partitions,
    replicated across cores' layout: w[p, s] = a[s*16 + p % 16]."""
    n = a.shape[0]
    w = a.reshape(n // 16, 16).T.astype(np.int16)  # [16, n//16]
    return np.tile(w, (8, 1))  # [128, n//16]


def _prepare(stock_features, stock_ids, interaction_matrix, attention_weights):
    X = np.asarray(stock_features, dtype=np.float32)
    ids = np.asarray(stock_ids).astype(np.int64)
    A = np.asarray(attention_weights, dtype=np.float32)
    M = np.asarray(interaction_matrix, dtype=np.float32)

    E = np.exp(np.abs(A))
    P = M * E
    PT2 = P.T.astype(ml_dtypes.bfloat16)           # PT2[v, u] = P[u, v]

    # Host softmax denominators and positional-diagonal corrections
    C = np.zeros((B, V), np.float32)
    for b in range(B):
        C[b] = np.bincount(ids[b], minlength=V)
    EC = E @ C.T                                   # [V, B]
    Ediag = np.ascontiguousarray(np.diagonal(E))
    Pdiag = np.ascontiguousarray(np.diagonal(P))
    bi = np.arange(B)[:, None]
    Z = EC[ids, bi] - Ediag[ids] + 1.0             # [B, N]
    rz = (1.0 / Z).astype(np.float32)
    d = Pdiag[ids].astype(np.float32)              # [B, N]

    # per-core u-column compaction to the union of its batches' ids
    ucols = []
    for c in range(NCORES):
        ucols.append(np.unique(ids[c * BPC:(c + 1) * BPC]))
    npc = NPC if max(len(u) for u in ucols) <= NPC else ((V + 127) // 128) * 128

    # counts[bb]: shared valid-index count for batch slot bb (max unique
    # count across cores, so the num_idxs_reg program immediate is SPMD-safe)
    uniq = [np.unique(ids[b], return_inverse=True) for b in range(B)]
    counts = [max(len(uniq[c * BPC + bb][0]) for c in range(NCORES))
              for bb in range(BPC)]

    xsc = np.zeros((B, 128, NKT * F), ml_dtypes.bfloat16)
    uidx = np.zeros((B, 128, 64), np.int16)
    for b in range(B):
        U, inv = uniq[b]
        XSc = np.zeros((N, F), np.float32)
        np.add.at(XSc, inv, X[b])
        xsc[b] = XSc.reshape(NKT, 128, F).transpose(1, 0, 2) \
                    .reshape(128, NKT * F).astype(ml_dtypes.bfloat16)
        cc = counts[b % BPC]
        Upad = np.full(N, -1, np.int64)   # negative tail: dma_gather skips
        Upad[:len(U)] = U
        Upad[len(U):cc] = 0               # benign valid pads up to the
        # shared count (gathered row 0, nullified by zero lhsT rows)
        for kt in range(NKT):
            uidx[b, :, kt * 8:(kt + 1) * 8] = _wrap16(
                Upad[kt * 128:(kt + 1) * 128]
            )

    in_maps = []
    colmaps = []
    for c in range(NCORES):
        b0 = c * BPC
        uc = ucols[c]
        ptc = np.zeros((V, npc), ml_dtypes.bfloat16)
        ptc[:, :len(uc)] = PT2[:, uc]
        # colmap[g] = position of id g in this core's compacted columns
        colmap = np.zeros(V, np.int64)
        colmap[uc] = np.arange(len(uc))
        colmaps.append(colmap)
        in_maps.append({
            "pt2": ptc,
            "xsc": np.ascontiguousarray(xsc[b0:b0 + BPC]),
            "uidx": np.ascontiguousarray(uidx[b0:b0 + BPC]),
        })
    return npc, counts, in_maps, colmaps, ids, X, rz, d


def _install_trace_shims():
    """The agent image lacks ``antenv.axon_hooks`` (the NTFF profile glue)
    and cloud artifact upload. Provide both so trace=True works."""
    import sys as _sys
    import types

    if "antenv.axon_hooks" not in _sys.modules:
        hook = None
        try:
            from trn_agent_boot.trn_boot import _ntff_profile_via_ctypes
            hook = _ntff_profile_via_ctypes("/opt/axon/libaxon_pjrt.so")
        except Exception as e:  # pragma: no cover
            print(f"ntff hook unavailable: {e}")
        mod = types.ModuleType("antenv.axon_hooks")
        mod._hook = hook
        mod.get_axon_ntff_profile_hook = lambda: mod._hook
        mod.set_axon_ntff_profile_hook = lambda h: setattr(mod, "_hook", h)
        _sys.modules["antenv.axon_hooks"] = mod
        try:
            import antenv
            antenv.axon_hooks = mod
        except Exception:
            pass

    import concourse.bass_utils as _bu
    _bu.upload_artifacts = lambda tmpdir: f"local://{tmpdir}"


def run(stock_features, stock_ids, interaction_matrix, attention_weights,
        trace=False, tmpdir=None):
    """Run the kernel; returns (output, BassKernelResults)."""
    if trace:
        _install_trace_shims()
    npc, counts, in_maps, colmaps, ids, X, rz, d = _prepare(
        stock_features, stock_ids, interaction_matrix, attention_weights
    )
    nc = _build_program(npc, counts)
    res = run_bass_kernel_spmd(
        nc, in_maps, list(range(NCORES)), trace=trace, tmpdir=tmpdir
    )
    # Host epilogue: out[b,i,f] = (out_vT[b][f, col(ids_i)] - d_i*X[b,i,f]) * rz_i
    out = np.empty((B, N, F), np.float32)
    for c in range(NCORES):
        ovT = res.results[c]["out"]                # [BPC, 128, npc] bf16
        cm = colmaps[c]
        for bb in range(BPC):
            b = c * BPC + bb
            g = ovT[bb][:, cm[ids[b]]].T.astype(np.float32)   # [N, F]
            out[b] = (g - d[b][:, None] * X[b]) * rz[b][:, None]
    return out, res


def kernel(stock_features, stock_ids, interaction_matrix, attention_weights):
    out, _ = run(stock_features, stock_ids, interaction_matrix,
                 attention_weights)
    return out


# revision 16
# speedup vs baseline: 1.0341x; 1.0341x over previous
"""Trainium2 Bass kernel for CointegrationAttentionLayer.

Reference computation (per batch b, ids = stock_ids[b], X = stock_features[b]):
    G_A[i,j] = attention_weights[ids_i, ids_j]   (0 on i==j diag)
    G_M[i,j] = interaction_matrix[ids_i, ids_j]  (0 on i==j diag)
    w = |G_A|; attn = softmax(w, axis=j)
    out[b] = (G_M * attn) @ X

Algebraic refactor (removes the on-chip column gather, the old bottleneck):
    E = exp(|A|), P = M * E                       (host: parameter folding)
    out[b,i] = (1/Z_i) * (sum_v P[ids_i, v] * XS_b[v]  -  P[ids_i,ids_i] * X[b,i])
    XS_b[v]  = sum_{j: ids_j = v} X[b,j]          (host scatter-add, O(N*F))
    Z[b,i]   = sum_v E[ids_i, v] * count_b[v] - E[ids_i,ids_i] + 1   (host)

Only the v in unique(ids_b) rows of XS_b are nonzero, so the device contracts
over K = 1024 padded unique ids:
    out_vT[f, u] = sum_k XSc_b[k, f] * PTc[U_b[k], u]
The device row-gathers PTc[U_b] via the 16-queue dma_gather (bf16 rows) and
runs dense accumulating matmuls into PSUM; no GpSimd ap_gather at all.
The u axis is compacted per core to the union of its 4 batches' ids
(~2563 of 4000, padded to NPC=2816), cutting gather traffic and matmul
stream length by ~31%.  Softmax normalization, the positional-diagonal
correction and the final row selection out_v[ids_i] are cheap O(B*N*F)
host index math on the returned [f, u] planes.

Sharding: data-parallel, 4 batches per core; per-core column-compacted table.
"""

import numpy as np
import ml_dtypes

import concourse.bacc as bacc
import concourse.bass as bass
import concourse.tile as tile
from concourse import mybir
from concourse.bass_utils import run_bass_kernel_spmd

B, N, F, V = 32, 1024, 128, 4000
NCORES = 8
BPC = B // NCORES    # batches per core
NKT = N // 128       # 8 contraction k-tiles per batch
NB = 512             # PSUM bank width in f32
NPC = 2816           # compacted/padded u-columns per core (22*128)

_prog_cache = {}


def _build_program(npc, counts):
    """counts[bb] = shared-across-cores valid index count for batch slot bb
    (indices beyond it are negative pads that dma_gather skips; the count is
    passed via num_idxs_reg so the decode-side descriptor-ring accounting
    stays in lockstep with the ucode's trailing-negative trim)."""
    key = (npc, tuple(counts))
    if key in _prog_cache:
        return _prog_cache[key]

    f32 = mybir.dt.float32
    bf16 = mybir.dt.bfloat16
    i16 = mybir.dt.int16

    nbanks = [NB] * (npc // NB)
    if npc % NB:
        nbanks.append(npc % NB)

    nc = bacc.Bacc(None, target_bir_lowering=False)
    pt2 = nc.declare_dram_parameter("pt2", [V, npc], bf16, isOutput=False)
    # xsc[b, p, kt*F + f] = XSc_b[kt*128 + p, f]
    xsc = nc.declare_dram_parameter("xsc", [BPC, 128, NKT * F], bf16,
                                    isOutput=False)
    # uidx[b] cols kt*8..kt*8+8 = wrapped int16 unique-id rows of k-tile kt
    uidx = nc.declare_dram_parameter("uidx", [BPC, 128, 64], i16,
                                     isOutput=False)
    out = nc.declare_dram_parameter("out", [BPC, 128, npc], bf16,
                                    isOutput=True)

    with tile.TileContext(nc) as tc, \
            tc.tile_pool(name="gat", bufs=2) as gatp, \
            tc.tile_pool(name="sml", bufs=1) as smlp, \
            tc.tile_pool(name="ob", bufs=4) as obp, \
            tc.tile_pool(name="psum", bufs=1, space="PSUM") as psump:
        uits, xsts = [], []
        for b in range(BPC):
            uit = smlp.tile([128, 64], i16, tag=f"uit{b}", name=f"uit{b}")
            nc.sync.dma_start(out=uit[:], in_=uidx[b])
            uits.append(uit)
        for b in range(BPC):
            xst = smlp.tile([128, NKT * F], bf16, tag=f"xst{b}",
                            name=f"xst{b}")
            nc.sync.dma_start(out=xst[:], in_=xsc[b])
            xsts.append(xst)

        for b in range(BPC):
            uit, xst = uits[b], xsts[b]
            bts = []
            for kt in range(NKT):
                bt = gatp.tile([128, npc], bf16, tag=f"bt{kt}", name=f"bt{kt}")
                # Trailing indices beyond counts[b] are negative: dma_gather
                # skips them (no DMA traffic); num_idxs_reg carries the valid
                # count so the decode-side ring accounting matches the ucode
                # trim.  Skipped partitions keep previous SBUF content, which
                # the zero lhsT rows nullify -- except the first two pool
                # generations are uninitialized SBUF (0 * NaN = NaN), so
                # memset just the partially-gathered tiles for b < 2.
                nvalid = max(0, min(128, counts[b] - kt * 128))
                if b < 2 and nvalid < 128:
                    nc.vector.memset(bt[:], 0.0)
                nc.gpsimd.dma_gather(
                    out_ap=bt[:].rearrange("p (o e) -> p o e", o=1),
                    in_ap=pt2[:],
                    idxs_ap=uit[:, kt * 8:(kt + 1) * 8],
                    num_idxs=128,
                    num_idxs_reg=nvalid,
                    elem_size=npc,
                )
                bts.append(bt)

            pos = [
                psump.tile([128, w], f32, tag=f"po{h}", name=f"po{h}",
                           space="PSUM")
                for h, w in enumerate(nbanks)
            ]
            for kt in range(NKT):
                for h, w in enumerate(nbanks):
                    nc.tensor.matmul(
                        out=pos[h][:],
                        lhsT=xst[:, kt * F:(kt + 1) * F],
                        rhs=bts[kt][:, h * NB:h * NB + w],
                        start=(kt == 0),
                        stop=(kt == NKT - 1),
                    )
            for h, w in enumerate(nbanks):
                ob = obp.tile([128, w], bf16, tag=f"ob{h}")
                nc.vector.tensor_copy(out=ob[:], in_=pos[h][:])
                eng = nc.scalar if h % 2 == 0 else nc.sync
                eng.dma_start(out=out[b, :, h * NB:h * NB + w], in_=ob[:])

    nc.compile()
    _prog_cache[key] = nc
    return nc


def _wrap16(a):
    """[n] int array -> [128, n//16] int16 'wrapped in 16 partitions,
    replicated across cores' layout: w[p, s] = a[s*16 + p % 16]."""
    n = a.shape[0]
    w = a.reshape(n // 16, 16).T.astype(np.int16)  # [16, n//16]
    return np.tile(w, (8, 1))  # [128, n//16]


def _prepare(stock_features, stock_ids, interaction_matrix, attention_weights):
    X = np.asarray(stock_features, dtype=np.float32)
    ids = np.asarray(stock_ids).astype(np.int64)
    A = np.asarray(attention_weights, dtype=np.float32)
    M = np.asarray(interaction_matrix, dtype=np.float32)

    E = np.exp(np.abs(A))
    P = M * E
    PT2 = P.T.astype(ml_dtypes.bfloat16)           # PT2[v, u] = P[u, v]

    # Host softmax denominators and positional-diagonal corrections
    C = np.zeros((B, V), np.float32)
    for b in range(B):
        C[b] = np.bincount(ids[b], minlength=V)
    EC = E @ C.T                                   # [V, B]
    Ediag = np.ascontiguousarray(np.diagonal(E))
    Pdiag = np.ascontiguousarray(np.diagonal(P))
    bi = np.arange(B)[:, None]
    Z = EC[ids, bi] - Ediag[ids] + 1.0             # [B, N]
    rz = (1.0 / Z).astype(np.float32)
    d = Pdiag[ids].astype(np.float32)              # [B, N]

    # per-core u-column compaction to the union of its batches' ids
    ucols = []
    for c in range(NCORES):
        ucols.append(np.unique(ids[c * BPC:(c + 1) * BPC]))
    npc = NPC if max(len(u) for u in ucols) <= NPC else ((V + 127) // 128) * 128

    # counts[bb]: shared valid-index count for batch slot bb (max unique
    # count across cores, so the num_idxs_reg program immediate is SPMD-safe)
    uniq = [np.unique(ids[b], return_inverse=True) for b in range(B)]
    counts = [max(len(uniq[c * BPC + bb][0]) for c in range(NCORES))
              for bb in range(BPC)]

    xsc = np.zeros((B, 128, NKT * F), ml_dtypes.bfloat16)
    uidx = np.zeros((B, 128, 64), np.int16)
    for b in range(B):
        U, inv = uniq[b]
        XSc = np.zeros((N, F), np.float32)
        np.add.at(XSc, inv, X[b])
        xsc[b] = XSc.reshape(NKT, 128, F).transpose(1, 0, 2) \
                    .reshape(128, NKT * F).astype(ml_dtypes.bfloat16)
        cc = counts[b % BPC]
        Upad = np.full(N, -1, np.int64)   # negative tail: dma_gather skips
        Upad[:len(U)] = U
        Upad[len(U):cc] = 0               # benign valid pads up to the
        # shared count (gathered row 0, nullified by zero lhsT rows)
        for kt in range(NKT):
            uidx[b, :, kt * 8:(kt + 1) * 8] = _wrap16(
                Upad[kt * 128:(kt + 1) * 128]
            )

    in_maps = []
    colmaps = []
    for c in range(NCORES):
        b0 = c * BPC
        uc = ucols[c]
        ptc = np.zeros((V, npc), ml_dtypes.bfloat16)
        ptc[:, :len(uc)] = PT2[:, uc]
        # colmap[g] = position of id g in this core's compacted columns
        colmap = np.zeros(V, np.int64)
        colmap[uc] = np.arange(len(uc))
        colmaps.append(colmap)
        in_maps.append({
            "pt2": ptc,
            "xsc": np.ascontiguousarray(xsc[b0:b0 + BPC]),
            "uidx": np.ascontiguousarray(uidx[b0:b0 + BPC]),
        })
    return npc, counts, in_maps, colmaps, ids, X, rz, d


def _install_trace_shims():
    """The agent image lacks ``antenv.axon_hooks`` (the NTFF profile glue)
    and cloud artifact upload. Provide both so trace=True works."""
    import sys as _sys
    import types

    if "antenv.axon_hooks" not in _sys.modules:
        hook = None
        try:
            from trn_agent_boot.trn_boot import _ntff_profile_via_ctypes
            hook = _ntff_profile_via_ctypes("/opt/axon/libaxon_pjrt.so")
        except Exception as e:  # pragma: no cover
            print(f"ntff hook unavailable: {e}")
        mod = types.ModuleType("antenv.axon_hooks")
        mod._hook = hook
        mod.get_axon_ntff_profile_hook = lambda: mod._hook
        mod.set_axon_ntff_profile_hook = lambda h: setattr(mod, "_hook", h)
        _sys.modules["antenv.axon_hooks"] = mod
        try:
            import antenv
            antenv.axon_hooks = mod
        except Exception:
            pass

    import concourse.bass_utils as _bu
    _bu.upload_artifacts = lambda tmpdir: f"local://{tmpdir}"


def run(stock_features, stock_ids, interaction_matrix, attention_weights,
        trace=False, tmpdir=None):
    """Run the kernel; returns (output, BassKernelResults)."""
    if trace:
        _install_trace_shims()
    npc, counts, in_maps, colmaps, ids, X, rz, d = _prepare(
        stock_features, stock_ids, interaction_matrix, attention_weights
    )
    nc = _build_program(npc, counts)
    res = run_bass_kernel_spmd(
        nc, in_maps, list(range(NCORES)), trace=trace, tmpdir=tmpdir
    )
    # Host epilogue: out[b,i,f] = (out_vT[b][f, col(ids_i)] - d_i*X[b,i,f]) * rz_i
    out = np.empty((B, N, F), np.float32)
    for c in range(NCORES):
        ovT = res.results[c]["out"]                # [BPC, 128, npc] bf16
        cm = colmaps[c]
        for bb in range(BPC):
            b = c * BPC + bb
            g = ovT[bb][:, cm[ids[b]]].T.astype(np.float32)   # [N, F]
            out[b] = (g - d[b][:, None] * X[b]) * rz[b][:, None]
    return out, res


def kernel(stock_features, stock_ids, interaction_matrix, attention_weights):
    out, _ = run(stock_features, stock_ids, interaction_matrix,
                 attention_weights)
    return out
